# revision 6
# baseline (speedup 1.0000x reference)
"""APPNP model on 8 TRN2 NeuronCores.

Math (reference):
    h   = relu(X @ W1 + b1)          X: dense [N,F] from COO features
    z   = h @ W2 + b2                [N, L]
    p   = propagator @ z             propagator: [N, N]  (1 GiB f32 -> memory bound)
    out = log_softmax(p, axis=1)

Distribution (8 cores, row-shard the propagator):
    core k owns rows rk = [k*R, (k+1)*R), R = N/8 = 2048
      - computes h_k, z_k for its own rows (X row-sharded, weights replicated)
      - AllGather z  (z is only [N,16] = 1 MiB -> cheap collective)
      - computes out^T[:, rk] = log_softmax(P[rk,:] @ z)^T
    Host feeds P^T slices (pt = P[rk,:].T, C-contiguous) so the contraction
    dim (columns of P) lands on the SBUF partition axis with fully
    sequential HBM reads; TensorE needs partition = contraction for both
    operands.

log_softmax note: p values are tiny (|p| < ~1 : P ~ U[0, 1/N), z logits
O(0.3)), so the max-subtraction in the reference log_softmax is a no-op
numerically; we compute p - log(sum_l exp(p_l)) directly. The sum over the
16 labels lives on the PSUM partition axis; it's done with a ones-vector
matmul, and the result is broadcast back across partitions with a second
ones matmul.
"""

import sys

for _p in ("/opt/trn_rl_repo",):
    if _p not in sys.path:
        sys.path.append(_p)

import numpy as np

import concourse.bacc as bacc
import concourse.bass as bass
import concourse.mybir as mybir
from concourse import tile
from concourse.bass_utils import run_bass_kernel_spmd

N = 16384          # nodes
F = 1024           # features
H = 64             # hidden
L = 16             # labels
NC = 8             # cores
R = N // NC        # rows per core = 2048
FCH = F // 128     # feature chunks = 8
ACH = N // 128     # contraction chunks for the big matmul = 128
RB = R // 512      # 512-wide moving slices per core = 4
ZCH = R // 128     # z row chunks per core = 16

F32 = mybir.dt.float32
BF16 = mybir.dt.bfloat16
P_DT = BF16        # propagator dtype on the wire/device
X_DT = BF16        # dense feature dtype on the wire/device
Z_DT = BF16        # latent logits dtype (gathered + stationary operand)
W_DT = BF16        # FC weight dtype (matmul operands must match moving dtype)

P_BUFS = 32        # SBUF double-buffer depth for 512 KiB propagator tiles


def _build_nc(N=N, F=F, H=H, L=L, NC=NC, P_DT=P_DT, X_DT=X_DT, Z_DT=Z_DT, W_DT=W_DT, P_BUFS=P_BUFS):
    R = N // NC
    FCH = F // 128
    ACH = N // 128
    RB = R // 512
    ZCH = R // 128
    nc = bacc.Bacc(None, target_bir_lowering=False, debug=False)

    pt = nc.dram_tensor("pt", [N, R], P_DT, kind="ExternalInput")    # P[rk,:].T
    xt = nc.dram_tensor("xt", [F, R], X_DT, kind="ExternalInput")    # X[rk,:].T
    w1 = nc.dram_tensor("w1", [F, H], W_DT, kind="ExternalInput")
    b1 = nc.dram_tensor("b1", [H, 1], F32, kind="ExternalInput")
    w2 = nc.dram_tensor("w2", [H, L], W_DT, kind="ExternalInput")
    b2r = nc.dram_tensor("b2r", [128, L], F32, kind="ExternalInput")  # b2 replicated
    out = nc.dram_tensor("out", [L, R], F32, kind="ExternalOutput")   # out^T slice

    with tile.TileContext(nc) as tc:
        with (
            tc.tile_pool(name="const", bufs=1) as const,
            tc.tile_pool(name="zpool", bufs=1) as zpool,
            tc.tile_pool(name="ppool", bufs=P_BUFS) as ppool,
            tc.tile_pool(name="dram", bufs=1, space="DRAM") as dram,
        ):
            # ---- constants -------------------------------------------------
            w1_s = const.tile([128, FCH, H], W_DT)
            nc.sync.dma_start(out=w1_s[:], in_=w1.rearrange("(a p) h -> p a h", p=128))
            b1_s = const.tile([H, 1], F32)
            nc.sync.dma_start(out=b1_s[:], in_=b1[:])
            w2_s = const.tile([H, L], W_DT)
            nc.sync.dma_start(out=w2_s[:], in_=w2[:])
            b2r_s = const.tile([128, L], F32)
            nc.sync.dma_start(out=b2r_s[:], in_=b2r[:])
            ones_col = const.tile([L, 1], F32)
            nc.gpsimd.memset(ones_col[:], 1.0)
            ones_row = const.tile([1, L], F32)
            nc.gpsimd.memset(ones_row[:], 1.0)

            with (
                tc.tile_pool(name="xpool", bufs=1) as xpool,
                tc.tile_pool(name="hpool", bufs=1) as hpool,
                tc.tile_pool(name="ps1", bufs=1, space="PSUM") as ps1,
                nc.named_scope("fc"),
            ):
                # ---- phase 1: h_k^T = relu(W1^T X_k^T + b1)  [64, R] ------
                xt_s = xpool.tile([128, FCH, R], X_DT)
                nc.sync.dma_start(out=xt_s[:],
                                  in_=xt.rearrange("(a p) j -> p a j", p=128))
                ph = ps1.tile([H, R], F32)
                for rb in range(RB):
                    sl = slice(rb * 512, (rb + 1) * 512)
                    for a in range(FCH):
                        nc.tensor.matmul(
                            ph[:, sl], w1_s[:, a, :], xt_s[:, a, sl],
                            start=(a == 0), stop=(a == FCH - 1),
                        )
                h_s = hpool.tile([H, R], Z_DT)
                nc.scalar.activation(h_s[:], ph[:],
                                     mybir.ActivationFunctionType.Relu,
                                     bias=b1_s[:])

                # ---- phase 2: z_k = h_k @ W2 + b2, laid out [R, L] --------
                z_s = zpool.tile([128, ZCH, L], Z_DT)
                for r in range(ZCH):
                    pz = ps1.tile([128, L], F32, tag="pz", bufs=2)
                    nc.tensor.matmul(pz[:], h_s[:, r * 128:(r + 1) * 128], w2_s[:])
                    nc.vector.tensor_add(z_s[:, r, :], pz[:], b2r_s[:])
                z_loc = dram.tile([R, L], Z_DT)
                nc.sync.dma_start(out=z_loc.rearrange("(r p) l -> p r l", p=128),
                                  in_=z_s[:])

            # ---- phase 3: AllGather z across the 8 cores ------------------
            z_all = dram.tile([N, L], Z_DT, addr_space="Shared")
            with nc.named_scope("gather"):
                nc.gpsimd.collective_compute(
                    "AllGather", mybir.AluOpType.bypass,
                    replica_groups=[list(range(NC))],
                    ins=[z_loc[:].opt()], outs=[z_all[:].opt()],
                )
            zt_s = zpool.tile([128, ACH, L], Z_DT)
            with nc.named_scope("ztload"):
                nc.sync.dma_start(out=zt_s[:],
                                  in_=z_all.rearrange("(a p) l -> p a l", p=128))

            with (
                tc.tile_pool(name="epool", bufs=2) as epool,
                tc.tile_pool(name="ps2", bufs=1, space="PSUM") as ps2,
            ):
                # ---- phase 4: out^T = z^T tiles @ P^T tiles (accumulate) --
                po = ps2.tile([L, R], F32)
                with nc.named_scope("prop"):
                    for a in range(ACH):
                        p_tile = ppool.tile([128, R], P_DT, tag="p_tile")
                        nc.sync.dma_start(out=p_tile[:],
                                          in_=pt[a * 128:(a + 1) * 128, :])
                        for rb in range(RB):
                            sl = slice(rb * 512, (rb + 1) * 512)
                            nc.tensor.matmul(
                                po[:, sl], zt_s[:, a, :], p_tile[:, sl],
                                start=(a == 0), stop=(a == ACH - 1),
                            )

                # ---- phase 5: log_softmax over the L=16 partition rows ----
                exp_s = epool.tile([L, R], F32, tag="e")
                nc.scalar.activation(exp_s[:], po[:],
                                     mybir.ActivationFunctionType.Exp)
                sum_p = ps2.tile([L, R], F32, tag="aux")
                for rb in range(RB):
                    sl = slice(rb * 512, (rb + 1) * 512)
                    nc.tensor.matmul(sum_p[:1, sl], ones_col[:], exp_s[:, sl])
                ls_s = epool.tile([1, R], F32)
                nc.scalar.activation(ls_s[:], sum_p[:1, :],
                                     mybir.ActivationFunctionType.Ln)
                rep_p = ps2.tile([L, R], F32, tag="aux")
                for rb in range(RB):
                    sl = slice(rb * 512, (rb + 1) * 512)
                    nc.tensor.matmul(rep_p[:, sl], ones_row[:], ls_s[:, sl])
                rep_s = epool.tile([L, R], F32, tag="e")
                nc.vector.tensor_copy(rep_s[:], rep_p[:])
                fin_s = epool.tile([L, R], F32, tag="e")
                nc.vector.tensor_sub(fin_s[:], po[:], rep_s[:])
                nc.sync.dma_start(out=out[:], in_=fin_s[:])

    nc.compile()
    return nc


_NC_CACHE = None


def _get_nc():
    global _NC_CACHE
    if _NC_CACHE is None:
        _NC_CACHE = _build_nc()
    return _NC_CACHE


def _densify(feature_indices, feature_values):
    rows = np.asarray(feature_indices[0]).astype(np.int64)
    cols = np.asarray(feature_indices[1]).astype(np.int64)
    vals = np.asarray(feature_values, dtype=np.float32)
    try:
        import scipy.sparse as sp
        X = np.asarray(
            sp.coo_matrix((vals, (rows, cols)), shape=(N, F)).todense(),
            dtype=np.float32)
    except ImportError:
        X = np.zeros((N, F), dtype=np.float32)
        np.add.at(X, (rows, cols), vals)
    return X


def kernel(feature_indices, feature_values, W1, b1, W2, b2, propagator):
    nc = _get_nc()

    X = _densify(feature_indices, feature_values)
    P = np.asarray(propagator, dtype=np.float32)
    w_np = mybir.dt.np(W_DT)
    W1 = np.asarray(W1, dtype=np.float32).astype(w_np)
    b1c = np.asarray(b1, dtype=np.float32).reshape(H, 1)
    W2 = np.asarray(W2, dtype=np.float32).astype(w_np)
    b2r = np.tile(np.asarray(b2, dtype=np.float32).reshape(1, L), (128, 1))
    b2r = np.ascontiguousarray(b2r)

    p_np = mybir.dt.np(P_DT)
    x_np = mybir.dt.np(X_DT)
    in_maps = []
    for k in range(NC):
        rk = slice(k * R, (k + 1) * R)
        in_maps.append({
            "pt": np.ascontiguousarray(P[rk, :].T).astype(p_np),
            "xt": np.ascontiguousarray(X[rk, :].T).astype(x_np),
            "w1": W1, "b1": b1c, "w2": W2, "b2r": b2r,
        })

    res = run_bass_kernel_spmd(nc, in_maps, list(range(NC)))
    out_full = np.empty((N, L), dtype=np.float32)
    for k in range(NC):
        out_full[k * R:(k + 1) * R, :] = res.results[k]["out"].T
    return out_full


# revision 10
# speedup vs baseline: 1.0440x; 1.0440x over previous
"""APPNP model on 8 TRN2 NeuronCores.

Math (reference):
    h   = relu(X @ W1 + b1)          X: dense [N,F] from COO features
    z   = h @ W2 + b2                [N, L]
    p   = propagator @ z             propagator: [N, N]  (1 GiB f32 -> memory bound)
    out = log_softmax(p, axis=1)

Distribution (8 cores, row-shard the propagator):
    core k owns rows rk = [k*R, (k+1)*R), R = N/8 = 2048
      - computes h_k, z_k for its own rows (X row-sharded, weights replicated)
      - AllGather z  (z is only [N,16] = 1 MiB -> cheap collective)
      - computes out^T[:, rk] = log_softmax(P[rk,:] @ z)^T
    Host feeds P^T slices (pt = P[rk,:].T, C-contiguous) so the contraction
    dim (columns of P) lands on the SBUF partition axis with fully
    sequential HBM reads; TensorE needs partition = contraction for both
    operands.

log_softmax note: p values are tiny (|p| < ~1 : P ~ U[0, 1/N), z logits
O(0.3)), so the max-subtraction in the reference log_softmax is a no-op
numerically; we compute p - log(sum_l exp(p_l)) directly. The sum over the
16 labels lives on the PSUM partition axis; it's done with a ones-vector
matmul, and the result is broadcast back across partitions with a second
ones matmul.
"""

import sys

for _p in ("/opt/trn_rl_repo",):
    if _p not in sys.path:
        sys.path.append(_p)

import numpy as np

import concourse.bacc as bacc
import concourse.bass as bass
import concourse.mybir as mybir
from concourse import tile
from concourse.bass_utils import run_bass_kernel_spmd

N = 16384          # nodes
F = 1024           # features
H = 64             # hidden
L = 16             # labels
NC = 8             # cores
R = N // NC        # rows per core = 2048
FCH = F // 128     # feature chunks = 8
ACH = N // 128     # contraction chunks for the big matmul = 128
RB = R // 512      # 512-wide moving slices per core = 4
ZCH = R // 128     # z row chunks per core = 16

F32 = mybir.dt.float32
BF16 = mybir.dt.bfloat16
P_DT = BF16        # propagator dtype on the wire/device
X_DT = BF16        # dense feature dtype on the wire/device
Z_DT = BF16        # latent logits dtype (gathered + stationary operand)
W_DT = BF16        # FC weight dtype (matmul operands must match moving dtype)

P_BUFS = 34        # SBUF double-buffer depth for 512 KiB propagator tiles


def _build_nc(N=N, F=F, H=H, L=L, NC=NC, P_DT=P_DT, X_DT=X_DT, Z_DT=Z_DT, W_DT=W_DT, P_BUFS=P_BUFS):
    R = N // NC
    FCH = F // 128
    ACH = N // 128
    RB = R // 512
    ZCH = R // 128
    nc = bacc.Bacc(None, target_bir_lowering=False, debug=False)

    pt = nc.dram_tensor("pt", [N, R], P_DT, kind="ExternalInput")    # P[rk,:].T
    xt = nc.dram_tensor("xt", [F, R], X_DT, kind="ExternalInput")    # X[rk,:].T
    w1 = nc.dram_tensor("w1", [F, H], W_DT, kind="ExternalInput")
    b1 = nc.dram_tensor("b1", [H, 1], F32, kind="ExternalInput")
    w2 = nc.dram_tensor("w2", [H, L], W_DT, kind="ExternalInput")
    b2r = nc.dram_tensor("b2r", [128, L], F32, kind="ExternalInput")  # b2 replicated
    ident = nc.dram_tensor("ident", [128, 128], Z_DT, kind="ExternalInput")
    out = nc.dram_tensor("out", [L, R], F32, kind="ExternalOutput")   # out^T slice

    with tile.TileContext(nc) as tc:
        with (
            tc.tile_pool(name="const", bufs=1) as const,
            tc.tile_pool(name="zpool", bufs=1) as zpool,
            tc.tile_pool(name="ppool", bufs=P_BUFS) as ppool,
            tc.tile_pool(name="dram", bufs=1, space="DRAM") as dram,
        ):
            # ---- constants -------------------------------------------------
            w1_s = const.tile([128, FCH, H], W_DT)
            nc.sync.dma_start(out=w1_s[:], in_=w1.rearrange("(a p) h -> p a h", p=128))
            b1_s = const.tile([H, 1], F32)
            nc.sync.dma_start(out=b1_s[:], in_=b1[:])
            w2_s = const.tile([H, L], W_DT)
            nc.sync.dma_start(out=w2_s[:], in_=w2[:])
            b2r_s = const.tile([128, L], F32)
            nc.sync.dma_start(out=b2r_s[:], in_=b2r[:])
            ones_col = const.tile([L, 1], F32)
            nc.gpsimd.memset(ones_col[:], 1.0)
            ones_row = const.tile([1, L], F32)
            nc.gpsimd.memset(ones_row[:], 1.0)
            id_s = const.tile([128, 128], Z_DT)
            nc.sync.dma_start(out=id_s[:], in_=ident[:])

            with (
                tc.tile_pool(name="xpool", bufs=1) as xpool,
                tc.tile_pool(name="hpool", bufs=1) as hpool,
                tc.tile_pool(name="ps1", bufs=1, space="PSUM") as ps1,
                nc.named_scope("fc"),
            ):
                # ---- phase 1: h_k^T = relu(W1^T X_k^T + b1)  [64, R] ------
                xt_s = xpool.tile([128, FCH, R], X_DT)
                xt_r = xt.rearrange("(a p) j -> a p j", p=128)
                for a in range(FCH):
                    nc.sync.dma_start(out=xt_s[:, a, :], in_=xt_r[a])
                ph = ps1.tile([H, R], F32)
                for rb in range(RB):
                    sl = slice(rb * 512, (rb + 1) * 512)
                    for a in range(FCH):
                        nc.tensor.matmul(
                            ph[:, sl], w1_s[:, a, :], xt_s[:, a, sl],
                            start=(a == 0), stop=(a == FCH - 1),
                        )
                h_s = hpool.tile([H, R], Z_DT)
                nc.scalar.activation(h_s[:], ph[:],
                                     mybir.ActivationFunctionType.Relu,
                                     bias=b1_s[:])

                # ---- phase 2: z_k = h_k @ W2 + b2, laid out [R, L] --------
                z_s = zpool.tile([128, ZCH, L], Z_DT)
                for r in range(ZCH):
                    pz = ps1.tile([128, L], F32, tag="pz", bufs=2)
                    nc.tensor.matmul(pz[:], h_s[:, r * 128:(r + 1) * 128], w2_s[:])
                    nc.vector.tensor_add(z_s[:, r, :], pz[:], b2r_s[:])
                z_loc = dram.tile([R, L], Z_DT)
                nc.sync.dma_start(out=z_loc.rearrange("(r p) l -> p r l", p=128),
                                  in_=z_s[:])

            # ---- phase 3: AllGather z across the 8 cores ------------------
            z_all = dram.tile([N, L], Z_DT, addr_space="Shared")
            with nc.named_scope("gather"):
                nc.gpsimd.collective_compute(
                    "AllGather", mybir.AluOpType.bypass,
                    replica_groups=[list(range(NC))],
                    ins=[z_loc[:].opt()], outs=[z_all[:].opt()],
                )
            # contiguous reload of the gathered z, then PE-transpose per label:
            # zraw[g, x, l] = z[g*128+x, l]; transpose (g,x) -> zt[x, g, l]
            zraw = zpool.tile([ACH, 128, L], Z_DT)
            with nc.named_scope("ztload"):
                nc.sync.dma_start(out=zraw[:],
                                  in_=z_all.rearrange("(g x) l -> g x l", x=128))
            zt_s = zpool.tile([128, ACH, L], Z_DT)

            with tc.tile_pool(name="pst", bufs=2, space="PSUM") as pst:
                with nc.named_scope("ztt"):
                    for l in range(L):
                        tp = pst.tile([128, ACH], Z_DT, tag="tp")
                        nc.tensor.transpose(tp[:], zraw[:, :, l], id_s[:ACH, :ACH])
                        nc.vector.tensor_copy(zt_s[:, :, l], tp[:])

            with (
                tc.tile_pool(name="epool", bufs=2) as epool,
                tc.tile_pool(name="ps2", bufs=1, space="PSUM") as ps2,
            ):
                # ---- phase 4: out^T = z^T tiles @ P^T tiles (accumulate) --
                po = ps2.tile([L, R], F32)
                with nc.named_scope("prop"):
                    for a in range(ACH):
                        p_tile = ppool.tile([128, R], P_DT, tag="p_tile")
                        nc.sync.dma_start(out=p_tile[:],
                                          in_=pt[a * 128:(a + 1) * 128, :])
                        for rb in range(RB):
                            sl = slice(rb * 512, (rb + 1) * 512)
                            nc.tensor.matmul(
                                po[:, sl], zt_s[:, a, :], p_tile[:, sl],
                                start=(a == 0), stop=(a == ACH - 1),
                            )

                # ---- phase 5: log_softmax over the L=16 partition rows ----
                exp_s = epool.tile([L, R], F32, tag="e")
                nc.scalar.activation(exp_s[:], po[:],
                                     mybir.ActivationFunctionType.Exp)
                sum_p = ps2.tile([L, R], F32, tag="aux")
                for rb in range(RB):
                    sl = slice(rb * 512, (rb + 1) * 512)
                    nc.tensor.matmul(sum_p[:1, sl], ones_col[:], exp_s[:, sl])
                ls_s = epool.tile([1, R], F32)
                nc.scalar.activation(ls_s[:], sum_p[:1, :],
                                     mybir.ActivationFunctionType.Ln)
                rep_p = ps2.tile([L, R], F32, tag="aux")
                for rb in range(RB):
                    sl = slice(rb * 512, (rb + 1) * 512)
                    nc.tensor.matmul(rep_p[:, sl], ones_row[:], ls_s[:, sl])
                rep_s = epool.tile([L, R], F32, tag="e")
                nc.vector.tensor_copy(rep_s[:], rep_p[:])
                fin_s = epool.tile([L, R], F32, tag="e")
                nc.vector.tensor_sub(fin_s[:], po[:], rep_s[:])
                nc.sync.dma_start(out=out[:], in_=fin_s[:])

    nc.compile()
    return nc


_NC_CACHE = None


def _get_nc():
    global _NC_CACHE
    if _NC_CACHE is None:
        _NC_CACHE = _build_nc()
    return _NC_CACHE


def _densify(feature_indices, feature_values):
    rows = np.asarray(feature_indices[0]).astype(np.int64)
    cols = np.asarray(feature_indices[1]).astype(np.int64)
    vals = np.asarray(feature_values, dtype=np.float32)
    try:
        import scipy.sparse as sp
        X = np.asarray(
            sp.coo_matrix((vals, (rows, cols)), shape=(N, F)).todense(),
            dtype=np.float32)
    except ImportError:
        X = np.zeros((N, F), dtype=np.float32)
        np.add.at(X, (rows, cols), vals)
    return X


def kernel(feature_indices, feature_values, W1, b1, W2, b2, propagator):
    nc = _get_nc()

    X = _densify(feature_indices, feature_values)
    P = np.asarray(propagator, dtype=np.float32)
    w_np = mybir.dt.np(W_DT)
    W1 = np.asarray(W1, dtype=np.float32).astype(w_np)
    b1c = np.asarray(b1, dtype=np.float32).reshape(H, 1)
    W2 = np.asarray(W2, dtype=np.float32).astype(w_np)
    b2r = np.tile(np.asarray(b2, dtype=np.float32).reshape(1, L), (128, 1))
    b2r = np.ascontiguousarray(b2r)
    idm = np.eye(128, dtype=np.float32).astype(mybir.dt.np(Z_DT))

    p_np = mybir.dt.np(P_DT)
    x_np = mybir.dt.np(X_DT)
    in_maps = []
    for k in range(NC):
        rk = slice(k * R, (k + 1) * R)
        in_maps.append({
            "pt": np.ascontiguousarray(P[rk, :].T).astype(p_np),
            "xt": np.ascontiguousarray(X[rk, :].T).astype(x_np),
            "w1": W1, "b1": b1c, "w2": W2, "b2r": b2r, "ident": idm,
        })

    res = run_bass_kernel_spmd(nc, in_maps, list(range(NC)))
    out_full = np.empty((N, L), dtype=np.float32)
    for k in range(NC):
        out_full[k * R:(k + 1) * R, :] = res.results[k]["out"].T
    return out_full
